# revision 5
# baseline (speedup 1.0000x reference)
"""Trainium2 Bass kernel for KeypointLoss — v2 (contiguous plane-aligned loads).

Full inputs:
  combined_preds [16, 4, 22, 128, 128] f32
  heatmaps       [16, 11, 128, 128]    f32
  labels         [16, 11, 11]          f32
Outputs: heat_loss [16, 4] f32, label_loss [16, 4] f32.

Sharding: data parallel over batch — core i handles batches [2i, 2i+2).

Per-core layout (BL=2, S=4, G=8 groups, K=11, H=W=128):
  Each group g=(b,s) is loaded CONTIGUOUSLY (4KB/2KB per-partition chunks,
  ~305 GB/s vs ~245 GB/s for the 512B h-major gather):
    T1_g [128,1024] f32: partition p=(k=p//16, hb=p%16), free (j,w) j<8,
         h = 8*hb + j   (planes 0-7, 4KB contiguous per partition)
    T2_g [ 96, 512] f32: partition p=(k=8+p//32, hb=p%32), free (j,w) j<4,
         h = 4*hb + j   (planes 8-10, 2KB contiguous per partition)
  ht[b] is loaded in the same two layouts via SWDGE with inline f32->bf16
  cast (ht never feeds the argmax, so bf16 is safe).

  heat_loss = sum hm*(hm-2ht) + sum ht^2 via PE bf16 matmuls (12 column
  chunks per group accumulated in one PSUM [128,128] block per group; diag
  extracted once per group). hm bf16 casts: T1 on ACT, T2 on GPSIMD.

  argmax (must stay f32): DVE rowmax per (plane,h) row -> R1all [128,64] /
  R2all [96,32]; batched head: PE transpose -> seg-reduce(max over hb) ->
  PE transpose -> seg-reduce(max over j) -> plane max M [11,8] (k,g).
  x: mask=is_ge(R^T, bcast(M)) * h-const, seg-reduce over hb, one matmul
  over j -> X [11,8]. y: indirect-gather row (128k+x) of each plane from
  DRAM, compare vs M, dot with iota. Label tail entirely in [11,8] (k,g)
  layout; final per-group sums via ones/indicator matmuls.
"""

import sys

for _p in ("/opt/trn_rl_repo", "/root/.axon_site/_ro/trn_rl_repo"):
    if _p not in sys.path:
        sys.path.append(_p)

from contextlib import ExitStack

import numpy as np

B, S, K, H, W = 16, 4, 11, 128, 128
NCORES = 8
BL = B // NCORES          # 2
G = BL * S                # 8
PL = G * K                # 88
C2 = 2 * K                # 22

_CACHE = {}

# aux column map (f32 [128, NAUX])
_C_ID = 0            # [128,128] identity
_C_IO = 128          # [128,128] iota (value = col)
_C_ONE = 256         # [128,1] ones
_C_RB = 257          # [11,8] rbase: DRAM row of (plane k of group g, h=0)
_C_IB1 = 265         # [8,128]  IndB1[k,p] = 1[p//16==k]
_C_IB2 = 393         # [3,96]   IndB2[k,p] = 1[p//32==k]
_C_J1 = 489          # [64,8]   Jind1[(g,j),g'] = 1[g==g']
_C_J2 = 497          # [32,8]   Jind2[(g,j),g'] = 1[g==g']
_C_H1 = 505          # [64,128] H1[(g,j),p] = 8*(p%16) + j
_C_H2 = 633          # [32,96]  H2[(g,j),p] = 4*(p%32) + j
_C_S8 = 729          # [8,11]   Sel8[r,c] = 1[c==r]
_C_S3 = 740          # [3,11]   Sel3[r,c] = 1[c==8+r]
_C_SEL88 = 751       # [11,88]  Sel88[p,i] = 1[p == i%11]
_C_OH8 = 839         # [88,8]   OneHot8[i,c] = 1[c == i%8]
_C_RB88 = 847        # [88,1]   rb88[i] = DRAM row of (plane k=i//8 of group g=i%8, h=0)
_C_KM = 848          # [88,8]   km[i,g] = 1[i%8 == g]  (plane -> group indicator)
_C_J96 = 856         # [96,88]  J96[p,i] = 1[g(p) == i//11]
_C_OH11 = 944        # [88,11]  OneHot11[i,c] = 1[c == i%11]
_C_SEL3B = 955       # [3,88]   Sel3B[p,i] = 1[p == i%11 - 8]
NAUX = 1043


def _aux_np():
    a = np.zeros((128, NAUX), np.float32)
    a[:, _C_ID:_C_ID + 128] = np.eye(128, dtype=np.float32)
    a[:, _C_IO:_C_IO + 128] = np.arange(128, dtype=np.float32)[None, :]
    a[:, _C_ONE] = 1.0
    for k in range(K):
        for g in range(G):
            b, s = divmod(g, S)
            a[k, _C_RB + g] = ((b * S + s) * C2 + k) * H
    for k in range(8):
        a[k, _C_IB1 + 16 * k:_C_IB1 + 16 * (k + 1)] = 1.0
    for k in range(3):
        a[k, _C_IB2 + 32 * k:_C_IB2 + 32 * (k + 1)] = 1.0
    for r in range(64):
        a[r, _C_J1 + r // 8] = 1.0
    for r in range(32):
        a[r, _C_J2 + r // 4] = 1.0
    for r in range(64):
        a[r, _C_H1:_C_H1 + 128] = 8.0 * (np.arange(128) % 16) + (r % 8)
    for r in range(32):
        a[r, _C_H2:_C_H2 + 96] = 4.0 * (np.arange(96) % 32) + (r % 4)
    for r in range(8):
        a[r, _C_S8 + r] = 1.0
    for r in range(3):
        a[r, _C_S3 + 8 + r] = 1.0
    for i in range(88):
        g, k = i // 11, i % 11
        b, s = divmod(g, S)
        a[k, _C_SEL88 + i] = 1.0
        a[i, _C_OH8 + g] = 1.0
        a[i, _C_RB88] = ((b * S + s) * C2 + k) * H
        a[i, _C_KM + g] = 1.0
        a[i, _C_OH11 + k] = 1.0
        if k >= 8:
            a[k - 8, _C_SEL3B + i] = 1.0
    for p in range(96):
        gp = p // 8 if p < 64 else (p - 64) // 4
        for i in range(88):
            if i // 11 == gp:
                a[p, _C_J96 + i] = 1.0
    return a


def _build_module(loop_n=1):
    import concourse.bass as bass
    import concourse.tile as tile
    from concourse import bacc, mybir

    f32 = mybir.dt.float32
    bf16 = mybir.dt.bfloat16
    u32 = mybir.dt.uint32
    Alu = mybir.AluOpType
    Act = mybir.ActivationFunctionType
    Ax = mybir.AxisListType

    nc = bacc.Bacc("TRN2", debug=False, enable_asserts=False, num_devices=1)

    cp = nc.dram_tensor("cp", [BL, S, C2, H, W], f32, kind="ExternalInput").ap()
    hmr = nc.dram_tensor("hmr", [BL, K, H, W], f32, kind="ExternalInput").ap()
    lbl = nc.dram_tensor("lbl", [BL, K, 11], f32, kind="ExternalInput").ap()
    out_heat = nc.dram_tensor("out_heat", [1, G], f32, kind="ExternalOutput").ap()
    out_label = nc.dram_tensor("out_label", [1, G], f32, kind="ExternalOutput").ap()
    import os as _os
    _dbg = _os.environ.get("K2DBG", "0") == "1"
    _skip = set(_os.environ.get("K2SKIP", "").split(","))
    if _dbg:
        out_mx = nc.dram_tensor("out_mx", [K, 16], f32, kind="ExternalOutput").ap()
        out_y = nc.dram_tensor("out_y", [K, G], f32, kind="ExternalOutput").ap()
        out_r1 = nc.dram_tensor("out_r1", [128, 64], f32, kind="ExternalOutput").ap()
        out_gath = nc.dram_tensor("out_gath", [K, G * 128], f32, kind="ExternalOutput").ap()

    aux_c = nc.inline_tensor(_aux_np(), "auxc").ap()
    APc = type(lbl)

    def t1_src(t, b, s_or_k):
        # t: cp group (b,s) or hmr (b,) ; planes 0..7
        src = cp[b, s_or_k, 0:8] if t == "hm" else hmr[b, 0:8]
        return src.rearrange("k (hb j) w -> (k hb) (j w)", hb=16, j=8)

    def t2_src(t, b, s_or_k):
        src = cp[b, s_or_k, 8:K] if t == "hm" else hmr[b, 8:K]
        return src.rearrange("k (hb j) w -> (k hb) (j w)", hb=32, j=4)

    with tile.TileContext(nc) as tc, ExitStack() as ctx:
        sb = ctx.enter_context(tc.tile_pool(name="sb", bufs=2))
        hmql = ctx.enter_context(tc.tile_pool(name="hmql", bufs=2))
        scr = ctx.enter_context(tc.tile_pool(name="scr", bufs=3))
        cst = ctx.enter_context(tc.tile_pool(name="cst", bufs=1))
        ps = ctx.enter_context(tc.tile_pool(name="ps", bufs=2, space="PSUM"))

        def emit():
            aux_t = sb.tile([128, NAUX], f32, name="aux_t")
            id_t = aux_t[:, _C_ID:_C_ID + 128]
            io_t = aux_t[:, _C_IO:_C_IO + 128]
            on_t = aux_t[:, _C_ONE:_C_ONE + 1]
            ib1_t = aux_t[0:8, _C_IB1:_C_IB1 + 128]
            ib2_t = aux_t[0:3, _C_IB2:_C_IB2 + 96]
            j1_t = aux_t[0:64, _C_J1:_C_J1 + 8]
            j2_t = aux_t[0:32, _C_J2:_C_J2 + 8]
            h1_t = aux_t[0:64, _C_H1:_C_H1 + 128]
            h2_t = aux_t[0:32, _C_H2:_C_H2 + 96]
            s8_t = aux_t[0:8, _C_S8:_C_S8 + 11]
            s3_t = aux_t[0:3, _C_S3:_C_S3 + 11]
            sel88_t = aux_t[0:K, _C_SEL88:_C_SEL88 + 88]
            j96_t = aux_t[0:96, _C_J96:_C_J96 + 88]
            oh11_t = aux_t[0:88, _C_OH11:_C_OH11 + 11]
            oh8_t = aux_t[0:88, _C_OH8:_C_OH8 + 8]
            rb88_t = aux_t[0:88, _C_RB88:_C_RB88 + 1]
            km_t = aux_t[0:88, _C_KM:_C_KM + 8]

            # ---- hm loads first: they gate the serial argmax chain ----
            T1, T2 = [], []
            t1_insts = []
            for g in range(G):
                b, s = divmod(g, S)
                t = hmql.tile([128, 1024], f32, name=f"T1_{g}")
                t1_insts.append(nc.sync.dma_start(t[:], t1_src("hm", b, s)))
                T1.append(t)
            nc.sync.dma_start(aux_t[:], aux_c)
            _t2q = _os.environ.get("K2T2Q", "sync")
            for g in range(G):
                b, s = divmod(g, S)
                t = hmql.tile([96, 512], f32, name=f"T2_{g}")
                (nc.gpsimd if _t2q == "gp" else nc.sync).dma_start(
                    t[:], t2_src("hm", b, s))
                T2.append(t)

            # ht (SWDGE inline f32->bf16): delayed behind the first T1 loads so
            # the hm stream keeps full HBM bandwidth early
            hT1, hT2 = [], []
            for b in range(BL):
                t1h = sb.tile([128, 1024], bf16, name=f"hT1_{b}")
                ih = nc.gpsimd.dma_start(t1h[:], t1_src("ht", b, None))
                _htd = int(_os.environ.get("K2HTD", "0"))
                if _htd >= 0:
                    tile.add_dep_helper(ih.ins, t1_insts[_htd].ins, sync=False,
                                        reason="delay ht behind early T1 loads")
                hT1.append(t1h)
            for b in range(BL):
                t2h = sb.tile([96, 512], bf16, name=f"hT2_{b}")
                nc.gpsimd.dma_start(t2h[:], t2_src("ht", b, None))
                hT2.append(t2h)

            # ---- small loads ----
            lblr = sb.tile([88, 11], f32, name="lblr")
            for b in range(BL):
                nc.sync.dma_start(
                    lblr[b * S * K:(b + 1) * S * K, :],
                    APc(lbl.tensor, b * K * 11, [[0, S], [11, K], [1, 11]]),
                )
            pred9 = sb.tile([88, 9], f32, name="pred9")
            nc.sync.dma_start(pred9[:], cp[:, :, K:C2, 0, 0:9])

            zrow = sb.tile([1, 512], bf16, name="zrow")
            nc.vector.memset(zrow[:], 0.0)

            # ---- per-group T1: rowmax (DVE) + cast*(-1/2) (ACT) ----
            R1all = sb.tile([128, 64], f32, name="R1all")
            R2all = sb.tile([96, 32], f32, name="R2all")
            C1, C2t = [], []
            do_rmax = "rowmax" not in _skip
            if not do_rmax:
                nc.vector.memset(R1all[:], 0.0)
                nc.vector.memset(R2all[:], 0.0)
            for g in range(G):
                if do_rmax:
                    nc.vector.tensor_reduce(
                        out=R1all[:, 8 * g:8 * (g + 1)],
                        in_=T1[g][:].rearrange("p (j w) -> p j w", j=8),
                        axis=Ax.X, op=Alu.max,
                    )
                c1 = (cst.tile([128, 1024], bf16, name=f"C1_{g}")
                      if _os.environ.get("K2CPOOL", "cst") == "cst"
                      else scr.tile([128, 1024], bf16, name=f"C1_{g}", tag="c1"))
                if "cast" not in _skip:
                    nc.scalar.mul(c1[:], T1[g][:], -0.5)
                C1.append(c1)
            for g in range(G):
                c2 = (cst.tile([96, 512], bf16, name=f"C2_{g}")
                      if _os.environ.get("K2CPOOL", "cst") == "cst"
                      else scr.tile([96, 512], bf16, name=f"C2_{g}", tag="c2"))
                if "cast" not in _skip:
                    nc.scalar.mul(c2[:], T2[g][:], -0.5)
                C2t.append(c2)

            # ---- label-tail prep that only needs lblr/pred9 (cheap, early) ----
            cdiff = sb.tile([88, 7], f32, name="cdiff")
            nc.vector.tensor_tensor(
                out=cdiff[:], in0=pred9[:, 0:7], in1=lblr[:, 0:7], op=Alu.subtract)
            t1 = sb.tile([88, 1], f32, name="t1")
            nc.vector.tensor_tensor(t1[:], lblr[:, 9:10], lblr[:, 7:8], Alu.add)
            t3 = sb.tile([88, 1], f32, name="t3")
            nc.vector.tensor_tensor(t3[:], lblr[:, 10:11], lblr[:, 8:9], Alu.add)
            gmin = sb.tile([88, 1], f32, name="gmin")
            nc.vector.tensor_tensor(gmin[:], lblr[:, 9:10], lblr[:, 10:11], Alu.min)
            gmax = sb.tile([88, 1], f32, name="gmax")
            nc.vector.tensor_tensor(gmax[:], lblr[:, 9:10], lblr[:, 10:11], Alu.max)
            c1v = sb.tile([88, 1], f32, name="c1v")
            nc.vector.tensor_scalar(c1v[:], gmin[:], 0.0, None, Alu.is_gt)
            c2v = sb.tile([88, 1], f32, name="c2v")
            nc.vector.tensor_scalar(c2v[:], gmax[:], float(H), None, Alu.is_lt)
            vv = sb.tile([88, 1], f32, name="vv")
            nc.vector.tensor_tensor(vv[:], c1v[:], c2v[:], Alu.mult)

            # ---- heat-loss matmuls: psall_g += (1/4)hm^2 - (1/2)hm*ht ----
            do_mm = "mm" not in _skip
            psA = ps.tile([128, 512], f32, name="psA", tag="psA")
            psB = ps.tile([128, 512], f32, name="psB", tag="psB")
            nc.tensor.matmul(out=psA[:], lhsT=zrow[:, 0:128], rhs=zrow[:],
                             start=True, stop=False, skip_group_check=True)
            nc.tensor.matmul(out=psB[:], lhsT=zrow[:, 0:128], rhs=zrow[:],
                             start=True, stop=False, skip_group_check=True)
            pht = ps.tile([128, 256], f32, name="pht", tag="pht")
            nc.tensor.matmul(out=pht[:], lhsT=zrow[:, 0:128], rhs=zrow[:, 0:256],
                             start=True, stop=False, skip_group_check=True)

            def ps_g(g):
                t = psA if g < 4 else psB
                return t[:, 128 * (g % 4):128 * (g % 4 + 1)]

            def group_mms(g):
                b = g // S
                first = None
                for c in range(8):
                    sl = slice(128 * c, 128 * (c + 1))
                    i0 = nc.tensor.matmul(out=ps_g(g), lhsT=C1[g][:, sl],
                                          rhs=C1[g][:, sl],
                                          start=False, stop=False,
                                          skip_group_check=True)
                    if first is None:
                        first = i0
                    nc.tensor.matmul(out=ps_g(g), lhsT=hT1[b][:, sl],
                                     rhs=C1[g][:, sl],
                                     start=False, stop=False,
                                     skip_group_check=True)
                for c in range(4):
                    sl = slice(128 * c, 128 * (c + 1))
                    nc.tensor.matmul(out=ps_g(g), lhsT=C2t[g][:, sl],
                                     rhs=C2t[g][:, sl],
                                     start=False, stop=False,
                                     skip_group_check=True)
                    nc.tensor.matmul(out=ps_g(g), lhsT=hT2[b][:, sl],
                                     rhs=C2t[g][:, sl],
                                     start=False, stop=(c == 3),
                                     skip_group_check=True)
                return first

            acc_hm = sb.tile([128, G], f32, name="acc_hm")
            acc_ht = sb.tile([128, BL], f32, name="acc_ht")

            def diag_g(g):
                dsc = scr.tile([128, 128], f32, name=f"dsc{g}", tag="dsc")
                return nc.vector.scalar_tensor_tensor(
                    out=dsc[:], in0=ps_g(g), scalar=1.0, in1=id_t[:],
                    op0=Alu.bypass, op1=Alu.mult, accum_out=acc_hm[:, g:g + 1],
                )

            for g in range(6 if do_mm else 0):
                group_mms(g)

            for b in range(BL if do_mm else 0):
                for c in range(8):
                    nc.tensor.matmul(
                        out=pht[:, 128 * b:128 * (b + 1)],
                        lhsT=hT1[b][:, 128 * c:128 * (c + 1)],
                        rhs=hT1[b][:, 128 * c:128 * (c + 1)],
                        start=False, stop=False, skip_group_check=True,
                    )
                for c in range(4):
                    nc.tensor.matmul(
                        out=pht[:, 128 * b:128 * (b + 1)],
                        lhsT=hT2[b][:, 128 * c:128 * (c + 1)],
                        rhs=hT2[b][:, 128 * c:128 * (c + 1)],
                        start=False, stop=(b == BL - 1 and c == 3),
                        skip_group_check=True,
                    )


            # ---- R2 rowmaxes (all groups, ahead of any head DVE ops) ----
            for g in range(G if do_rmax else 0):
                nc.vector.tensor_reduce(
                    out=R2all[:, 4 * g:4 * (g + 1)],
                    in_=T2[g][:].rearrange("p (j w) -> p j w", j=4),
                    axis=Ax.X, op=Alu.max,
                )

            # ---- T1-part of the argmax head (overlaps late T2 loads) ----
            head1 = ps.tile([128, 512], f32, name="head1", tag="head1")
            head2 = head1
            do_head = "head" not in _skip
            rt1 = head1[0:64, 0:128]
            MX1 = sb.tile([8, 8], f32, name="MX1")    # plane max [k<8, g]
            MX2 = sb.tile([3, 8], f32, name="MX2")    # plane max [k-8, g]
            xp = sb.tile([96, 11], f32, name="xp")    # cols 0:8 T1-k, 8:11 T2-k
            nc.vector.memset(xp[:], 0.0)
            if do_head:
                nc.tensor.transpose(out=rt1, in_=R1all[:], identity=id_t[:])
                rt1s = sb.tile([64, 128], f32, name="rt1s")
                nc.scalar.copy(rt1s[:], rt1)
                V1 = sb.tile([64, 8], f32, name="V1")
                nc.vector.tensor_reduce(
                    out=V1[:], in_=rt1.rearrange("p (k hb) -> p k hb", k=8),
                    axis=Ax.X, op=Alu.max)
                vt1 = head1[0:8, 128:192]
                nc.tensor.transpose(out=vt1, in_=V1[:], identity=id_t[0:64, 0:64])
                nc.vector.tensor_reduce(
                    out=MX1[:], in_=vt1.rearrange("p (g j) -> p g j", g=8),
                    axis=Ax.X, op=Alu.max)
                m1rep = sb.tile([8, 64], f32, name="m1rep")
                nc.vector.tensor_copy(
                    out=m1rep[:].rearrange("k (g j) -> k g j", g=8),
                    in_=MX1[:].rearrange("k (g o) -> k g o", o=1)
                        .to_broadcast([8, 8, 8]),
                )
                mbc1 = head1[0:64, 224:352]
                nc.tensor.matmul(out=mbc1, lhsT=m1rep[:], rhs=ib1_t, start=True,
                                 stop=True)
                mk1 = scr.tile([64, 128], f32, name="mk1", tag="mk")
                nc.vector.tensor_tensor(out=mk1[:], in0=rt1s[:], in1=mbc1,
                                        op=Alu.is_ge)
                mh1 = scr.tile([64, 128], f32, name="mh1", tag="mh")
                nc.vector.tensor_tensor(out=mh1[:], in0=mk1[:], in1=h1_t,
                                        op=Alu.mult)
                nc.vector.tensor_reduce(
                    out=xp[0:64, 0:8],
                    in_=mh1[:].rearrange("p (k hb) -> p k hb", k=8),
                    axis=Ax.X, op=Alu.add)

            # ---- late groups: MMs + R2 + T2-head ----
            late_firsts = []
            _mmd = int(_os.environ.get("K2MMD", "6"))
            for g in range(6, 8):
                if do_mm:
                    fi = group_mms(g)
                    if g >= _mmd:
                        late_firsts.append(fi)

            if do_head:
                rt2 = head2[0:32, 0:96]
                nc.tensor.transpose(out=rt2, in_=R2all[:],
                                    identity=id_t[0:96, 0:96])
                rt2s = sb.tile([32, 96], f32, name="rt2s")
                nc.scalar.copy(rt2s[:], rt2)
                V2 = sb.tile([32, 3], f32, name="V2")
                nc.vector.tensor_reduce(
                    out=V2[:], in_=rt2.rearrange("p (k hb) -> p k hb", k=3),
                    axis=Ax.X, op=Alu.max)
                vt2 = head2[0:3, 192:224]
                nc.tensor.transpose(out=vt2, in_=V2[:], identity=id_t[0:32, 0:32])
                nc.vector.tensor_reduce(
                    out=MX2[:], in_=vt2.rearrange("p (g j) -> p g j", g=8),
                    axis=Ax.X, op=Alu.max)
                m2rep = sb.tile([3, 32], f32, name="m2rep")
                nc.vector.tensor_copy(
                    out=m2rep[:].rearrange("k (g j) -> k g j", g=8),
                    in_=MX2[:].rearrange("k (g o) -> k g o", o=1)
                        .to_broadcast([3, 8, 4]),
                )
                mbc2 = head2[0:32, 352:448]
                nc.tensor.matmul(out=mbc2, lhsT=m2rep[:], rhs=ib2_t, start=True,
                                 stop=True)
                mk2 = scr.tile([32, 96], f32, name="mk2", tag="mk")
                nc.vector.tensor_tensor(out=mk2[:], in0=rt2s[:], in1=mbc2,
                                        op=Alu.is_ge)
                mh2 = scr.tile([32, 96], f32, name="mh2", tag="mh")
                nc.vector.tensor_tensor(out=mh2[:], in0=mk2[:], in1=h2_t,
                                        op=Alu.mult)
                nc.vector.tensor_reduce(
                    out=xp[64:96, 8:11],
                    in_=mh2[:].rearrange("p (k hb) -> p k hb", k=3),
                    axis=Ax.X, op=Alu.add)

            # class loss square (ACT; after the head copies so it doesn't
            # block rt1s/rt2s in the ACT queue)
            csc = sb.tile([88, 7], f32, name="csc")
            cls = sb.tile([88, 1], f32, name="cls")
            nc.scalar.activation(out=csc[:], in_=cdiff[:], func=Act.Square,
                                 accum_out=cls[:])

            # ---- scatter M and x directly onto 88 plane-partitions ----
            if not do_head:
                nc.vector.memset(MX1[:], 1.0)
                nc.vector.memset(MX2[:], 1.0)
            outM = head1[0:88, 448:456]
            nc.tensor.matmul(out=outM, lhsT=aux_t[0:8, _C_SEL88:_C_SEL88 + 88],
                             rhs=MX1[:], start=True, stop=False,
                             skip_group_check=True)
            nc.tensor.matmul(out=outM,
                             lhsT=aux_t[0:3, _C_SEL3B:_C_SEL3B + 88],
                             rhs=MX2[:], start=False, stop=True,
                             skip_group_check=True)
            outX = head1[0:88, 456:467]
            outx_inst = nc.tensor.matmul(out=outX, lhsT=j96_t, rhs=xp[:],
                                         start=True, stop=True,
                                         skip_group_check=True)
            for _fi in late_firsts:
                if _fi is not None and do_head:
                    tile.add_dep_helper(_fi.ins, outx_inst.ins, sync=False,
                                        reason="run argmax-head PE ops first")
            MX88 = sb.tile([88, 2], f32, name="MX88")
            sc88a = scr.tile([88, 8], f32, name="sc88a", tag="sc88")
            nc.vector.scalar_tensor_tensor(
                out=sc88a[:], in0=outM, scalar=1.0, in1=oh8_t,
                op0=Alu.bypass, op1=Alu.mult, accum_out=MX88[:, 0:1],
            )
            sc88b = scr.tile([88, 11], f32, name="sc88b", tag="sc88")
            nc.vector.scalar_tensor_tensor(
                out=sc88b[:], in0=outX, scalar=1.0, in1=oh11_t,
                op0=Alu.bypass, op1=Alu.mult, accum_out=MX88[:, 1:2],
            )
            ridu = sb.tile([88, 1], u32, name="ridu")
            ridu_inst = nc.vector.tensor_tensor(
                out=ridu[:], in0=MX88[:, 1:2], in1=rb88_t, op=Alu.add)
            gath = sb.tile([88, 128], f32, name="gath")
            if "gather" in _skip:
                nc.vector.memset(gath[:], 0.0)
            else:
                nc.gpsimd.indirect_dma_start(
                    out=gath[:],
                    out_offset=None,
                    in_=cp.rearrange("b s c h w -> (b s c h) w"),
                    in_offset=bass.IndirectOffsetOnAxis(ap=ridu[:, 0:1], axis=0),
                )
            conf = sb.tile([88, 1], f32, name="conf")
            nc.scalar.activation(out=conf[:], in_=MX88[:, 0:1], func=Act.Square,
                                 bias=1.0, scale=-1.0)

            # ---- diags: late groups deferred into the gather window; early
            # groups + ht left free for the scheduler to slot into gaps ----
            for g in range(G if do_mm else 0):
                d_inst = diag_g(g)
                if g >= int(_os.environ.get("K2DDEF", "4")):
                    tile.add_dep_helper(
                        d_inst.ins, ridu_inst.ins, sync=False,
                        reason="defer diag past argmax head")
            for b in range(BL if do_mm else 0):
                dht = scr.tile([128, 128], f32, name=f"dht{b}", tag="dsc")
                nc.vector.scalar_tensor_tensor(
                    out=dht[:], in0=pht[:, 128 * b:128 * (b + 1)], scalar=1.0,
                    in1=id_t[:], op0=Alu.bypass, op1=Alu.mult,
                    accum_out=acc_ht[:, b:b + 1],
                )
            if not do_mm:
                nc.vector.memset(acc_hm[:], 0.0)
                nc.vector.memset(acc_ht[:], 0.0)

            # ---- heat loss out ----
            acc_fin = sb.tile([128, G], f32, name="acc_fin")
            for b in range(BL):
                nc.vector.scalar_tensor_tensor(
                    out=acc_fin[:, b * S:(b + 1) * S],
                    in0=acc_hm[:, b * S:(b + 1) * S],
                    scalar=4.0,
                    in1=acc_ht[:, b:b + 1].to_broadcast([128, S]),
                    op0=Alu.mult,
                    op1=Alu.add,
                )
            psum_hs = head1[0:1, 467:475]
            nc.tensor.matmul(out=psum_hs, lhsT=on_t[:], rhs=acc_fin[:],
                             start=True, stop=True)
            heat_row = sb.tile([1, G], f32, name="heat_row")
            nc.vector.tensor_copy(out=heat_row[:], in_=psum_hs)
            nc.scalar.dma_start(out_heat, heat_row[:])

            # ---- y + rest of label tail ----
            yf = sb.tile([88, 1], f32, name="yf")
            ysc = scr.tile([88, 128], f32, name="ysc", tag="ysc")
            nc.vector.scalar_tensor_tensor(
                out=ysc[:], in0=gath[:], scalar=MX88[:, 0:1], in1=io_t[0:88, :],
                op0=Alu.is_equal, op1=Alu.mult, accum_out=yf[:],
            )
            t2 = sb.tile([88, 1], f32, name="t2")
            nc.vector.tensor_tensor(t2[:], MX88[:, 1:2], pred9[:, 7:8], Alu.add)
            tx = sb.tile([88, 1], f32, name="tx")
            nc.vector.tensor_tensor(tx[:], t1[:], t2[:], Alu.subtract)
            ccv = sb.tile([88, 1], f32, name="ccv")
            nc.vector.tensor_tensor(ccv[:], cls[:], conf[:], Alu.add)
            base = sb.tile([88, 1], f32, name="base")
            nc.vector.scalar_tensor_tensor(
                out=base[:], in0=tx[:], scalar=tx[:, 0:1], in1=ccv[:],
                op0=Alu.mult, op1=Alu.add,
            )
            t4 = sb.tile([88, 1], f32, name="t4")
            nc.vector.tensor_tensor(t4[:], yf[:], pred9[:, 8:9], Alu.add)
            ty = sb.tile([88, 1], f32, name="ty")
            nc.vector.tensor_tensor(ty[:], t3[:], t4[:], Alu.subtract)
            tot3 = sb.tile([88, 1], f32, name="tot3")
            nc.vector.scalar_tensor_tensor(
                out=tot3[:], in0=ty[:], scalar=ty[:, 0:1], in1=base[:],
                op0=Alu.mult, op1=Alu.add,
            )
            perkp = sb.tile([88, 1], f32, name="perkp")
            nc.vector.tensor_tensor(perkp[:], tot3[:], vv[:], Alu.mult)
            if _dbg:
                nc.sync.dma_start(out_r1, R1all[:])
            psum_lk = head2[0:1, 475:483]
            nc.tensor.matmul(out=psum_lk, lhsT=perkp[:], rhs=km_t, start=True,
                             stop=True)
            lab_row = sb.tile([1, G], f32, name="lab_row")
            nc.vector.tensor_copy(out=lab_row[:], in_=psum_lk)
            nc.sync.dma_start(out_label, lab_row[:])

        unroll = int(__import__("os").environ.get("K2UNROLL", "2"))
        if loop_n > 1 and loop_n >= 2 * unroll and unroll > 1:
            n2 = loop_n // unroll
            rem = loop_n - n2 * unroll
            with tc.For_i(0, n2, 1):
                for _u in range(unroll):
                    emit()
            for _u in range(rem):
                emit()
        elif loop_n > 1:
            with tc.For_i(0, loop_n, 1):
                emit()
        else:
            emit()

    nc.compile()
    return nc


def _get_nc(reps=1, loop_n=1):
    key = f"nc{reps}_{loop_n}"
    if key not in _CACHE:
        _CACHE[key] = _build_module(loop_n)
    return _CACHE[key]


def _in_maps(combined_preds, heatmaps, labels):
    cpv = np.ascontiguousarray(combined_preds, dtype=np.float32)
    hmv = np.ascontiguousarray(heatmaps, dtype=np.float32)
    lbv = np.ascontiguousarray(labels, dtype=np.float32)
    maps = []
    for i in range(NCORES):
        b0 = BL * i
        maps.append(
            {
                "cp": np.ascontiguousarray(cpv[b0:b0 + BL]),
                "hmr": np.ascontiguousarray(hmv[b0:b0 + BL]),
                "lbl": np.ascontiguousarray(lbv[b0:b0 + BL]),
            }
        )
    return maps


def run(combined_preds, heatmaps, labels, trace=False):
    from concourse import bass_utils

    nc = _get_nc()
    res = bass_utils.run_bass_kernel_spmd(
        nc,
        _in_maps(combined_preds, heatmaps, labels),
        core_ids=list(range(NCORES)),
        trace=trace,
    )
    # out_heat/out_label are [1, G] rows with G = BL*S in (b, s) order
    heat = np.concatenate(
        [res.results[i]["out_heat"].reshape(BL, S) for i in range(NCORES)], axis=0
    )
    lab = np.concatenate(
        [res.results[i]["out_label"].reshape(BL, S) for i in range(NCORES)], axis=0
    )
    return (heat, lab), res


def kernel(combined_preds, heatmaps, labels):
    (heat, lab), _ = run(combined_preds, heatmaps, labels)
    return heat, lab


# revision 6
# speedup vs baseline: 1.2418x; 1.2418x over previous
"""Trainium2 Bass kernel for KeypointLoss — v2 (contiguous plane-aligned loads).

Full inputs:
  combined_preds [16, 4, 22, 128, 128] f32
  heatmaps       [16, 11, 128, 128]    f32
  labels         [16, 11, 11]          f32
Outputs: heat_loss [16, 4] f32, label_loss [16, 4] f32.

Sharding: data parallel over batch — core i handles batches [2i, 2i+2).

Per-core layout (BL=2, S=4, G=8 groups, K=11, H=W=128):
  Each group g=(b,s) is loaded CONTIGUOUSLY (4KB/2KB per-partition chunks,
  ~305 GB/s vs ~245 GB/s for the 512B h-major gather):
    T1_g [128,1024] f32: partition p=(k=p//16, hb=p%16), free (j,w) j<8,
         h = 8*hb + j   (planes 0-7, 4KB contiguous per partition)
    T2_g [ 96, 512] f32: partition p=(k=8+p//32, hb=p%32), free (j,w) j<4,
         h = 4*hb + j   (planes 8-10, 2KB contiguous per partition)
  ht[b] is loaded in the same two layouts via SWDGE with inline f32->bf16
  cast (ht never feeds the argmax, so bf16 is safe).

  heat_loss = sum hm*(hm-2ht) + sum ht^2 via PE bf16 matmuls (12 column
  chunks per group accumulated in one PSUM [128,128] block per group; diag
  extracted once per group). hm bf16 casts: T1 on ACT, T2 on GPSIMD.

  argmax (must stay f32): DVE rowmax per (plane,h) row -> R1all [128,64] /
  R2all [96,32]; batched head: PE transpose -> seg-reduce(max over hb) ->
  PE transpose -> seg-reduce(max over j) -> plane max M [11,8] (k,g).
  x: mask=is_ge(R^T, bcast(M)) * h-const, seg-reduce over hb, one matmul
  over j -> X [11,8]. y: indirect-gather row (128k+x) of each plane from
  DRAM, compare vs M, dot with iota. Label tail entirely in [11,8] (k,g)
  layout; final per-group sums via ones/indicator matmuls.
"""

import sys

for _p in ("/opt/trn_rl_repo", "/root/.axon_site/_ro/trn_rl_repo"):
    if _p not in sys.path:
        sys.path.append(_p)

from contextlib import ExitStack

import numpy as np

B, S, K, H, W = 16, 4, 11, 128, 128
NCORES = 8
BL = B // NCORES          # 2
G = BL * S                # 8
PL = G * K                # 88
C2 = 2 * K                # 22

_CACHE = {}

# aux column map (f32 [128, NAUX])
_C_ID = 0            # [128,128] identity
_C_IO = 128          # [128,128] iota (value = col)
_C_ONE = 256         # [128,1] ones
_C_RB = 257          # [11,8] rbase: DRAM row of (plane k of group g, h=0)
_C_IB1 = 265         # [8,128]  IndB1[k,p] = 1[p//16==k]
_C_IB2 = 393         # [3,96]   IndB2[k,p] = 1[p//32==k]
_C_J1 = 489          # [64,8]   Jind1[(g,j),g'] = 1[g==g']
_C_J2 = 497          # [32,8]   Jind2[(g,j),g'] = 1[g==g']
_C_H1 = 505          # [64,128] H1[(g,j),p] = 8*(p%16) + j
_C_H2 = 633          # [32,96]  H2[(g,j),p] = 4*(p%32) + j
_C_S8 = 729          # [8,11]   Sel8[r,c] = 1[c==r]
_C_S3 = 740          # [3,11]   Sel3[r,c] = 1[c==8+r]
_C_SEL88 = 751       # [11,88]  Sel88[p,i] = 1[p == i%11]
_C_OH8 = 839         # [88,8]   OneHot8[i,c] = 1[c == i%8]
_C_RB88 = 847        # [88,1]   rb88[i] = DRAM row of (plane k=i//8 of group g=i%8, h=0)
_C_KM = 848          # [88,8]   km[i,g] = 1[i%8 == g]  (plane -> group indicator)
_C_J96 = 856         # [96,88]  J96[p,i] = 1[g(p) == i//11]
_C_OH11 = 944        # [88,11]  OneHot11[i,c] = 1[c == i%11]
_C_SEL3B = 955       # [3,88]   Sel3B[p,i] = 1[p == i%11 - 8]
NAUX = 1043


def _aux_np():
    a = np.zeros((128, NAUX), np.float32)
    a[:, _C_ID:_C_ID + 128] = np.eye(128, dtype=np.float32)
    a[:, _C_IO:_C_IO + 128] = np.arange(128, dtype=np.float32)[None, :]
    a[:, _C_ONE] = 1.0
    for k in range(K):
        for g in range(G):
            b, s = divmod(g, S)
            a[k, _C_RB + g] = ((b * S + s) * C2 + k) * H
    for k in range(8):
        a[k, _C_IB1 + 16 * k:_C_IB1 + 16 * (k + 1)] = 1.0
    for k in range(3):
        a[k, _C_IB2 + 32 * k:_C_IB2 + 32 * (k + 1)] = 1.0
    for r in range(64):
        a[r, _C_J1 + r // 8] = 1.0
    for r in range(32):
        a[r, _C_J2 + r // 4] = 1.0
    for r in range(64):
        a[r, _C_H1:_C_H1 + 128] = 8.0 * (np.arange(128) % 16) + (r % 8)
    for r in range(32):
        a[r, _C_H2:_C_H2 + 96] = 4.0 * (np.arange(96) % 32) + (r % 4)
    for r in range(8):
        a[r, _C_S8 + r] = 1.0
    for r in range(3):
        a[r, _C_S3 + 8 + r] = 1.0
    for i in range(88):
        g, k = i // 11, i % 11
        b, s = divmod(g, S)
        a[k, _C_SEL88 + i] = 1.0
        a[i, _C_OH8 + g] = 1.0
        a[i, _C_RB88] = ((b * S + s) * C2 + k) * H
        a[i, _C_KM + g] = 1.0
        a[i, _C_OH11 + k] = 1.0
        if k >= 8:
            a[k - 8, _C_SEL3B + i] = 1.0
    for p in range(96):
        gp = p // 8 if p < 64 else (p - 64) // 4
        for i in range(88):
            if i // 11 == gp:
                a[p, _C_J96 + i] = 1.0
    return a


def _build_module(loop_n=1):
    import concourse.bass as bass
    import concourse.tile as tile
    from concourse import bacc, mybir

    f32 = mybir.dt.float32
    bf16 = mybir.dt.bfloat16
    u32 = mybir.dt.uint32
    Alu = mybir.AluOpType
    Act = mybir.ActivationFunctionType
    Ax = mybir.AxisListType

    nc = bacc.Bacc("TRN2", debug=False, enable_asserts=False, num_devices=1)

    cp = nc.dram_tensor("cp", [BL, S, C2, H, W], f32, kind="ExternalInput").ap()
    hmr = nc.dram_tensor("hmr", [BL, K, H, W], f32, kind="ExternalInput").ap()
    lbl = nc.dram_tensor("lbl", [BL, K, 11], f32, kind="ExternalInput").ap()
    out_heat = nc.dram_tensor("out_heat", [1, G], f32, kind="ExternalOutput").ap()
    out_label = nc.dram_tensor("out_label", [1, G], f32, kind="ExternalOutput").ap()
    import os as _os
    _dbg = _os.environ.get("K2DBG", "0") == "1"
    _skip = set(_os.environ.get("K2SKIP", "").split(","))
    if _dbg:
        out_mx = nc.dram_tensor("out_mx", [K, 16], f32, kind="ExternalOutput").ap()
        out_y = nc.dram_tensor("out_y", [K, G], f32, kind="ExternalOutput").ap()
        out_r1 = nc.dram_tensor("out_r1", [128, 64], f32, kind="ExternalOutput").ap()
        out_gath = nc.dram_tensor("out_gath", [K, G * 128], f32, kind="ExternalOutput").ap()

    aux_c = nc.inline_tensor(_aux_np(), "auxc").ap()
    APc = type(lbl)

    def t1_src(t, b, s_or_k):
        # t: cp group (b,s) or hmr (b,) ; planes 0..7
        src = cp[b, s_or_k, 0:8] if t == "hm" else hmr[b, 0:8]
        return src.rearrange("k (hb j) w -> (k hb) (j w)", hb=16, j=8)

    def t2_src(t, b, s_or_k):
        src = cp[b, s_or_k, 8:K] if t == "hm" else hmr[b, 8:K]
        return src.rearrange("k (hb j) w -> (k hb) (j w)", hb=32, j=4)

    with tile.TileContext(nc) as tc, ExitStack() as ctx:
        sb = ctx.enter_context(tc.tile_pool(name="sb", bufs=2))
        hmql = ctx.enter_context(tc.tile_pool(name="hmql", bufs=2))
        scr = ctx.enter_context(tc.tile_pool(name="scr", bufs=3))
        cst = ctx.enter_context(tc.tile_pool(name="cst", bufs=1))
        axp = ctx.enter_context(tc.tile_pool(name="axp", bufs=1))
        ps = ctx.enter_context(tc.tile_pool(name="ps", bufs=2, space="PSUM"))

        aux_t = axp.tile([128, NAUX], f32, name="aux_t")
        nc.sync.dma_start(aux_t[:], aux_c)

        def emit():
            id_t = aux_t[:, _C_ID:_C_ID + 128]
            io_t = aux_t[:, _C_IO:_C_IO + 128]
            on_t = aux_t[:, _C_ONE:_C_ONE + 1]
            ib1_t = aux_t[0:8, _C_IB1:_C_IB1 + 128]
            ib2_t = aux_t[0:3, _C_IB2:_C_IB2 + 96]
            j1_t = aux_t[0:64, _C_J1:_C_J1 + 8]
            j2_t = aux_t[0:32, _C_J2:_C_J2 + 8]
            h1_t = aux_t[0:64, _C_H1:_C_H1 + 128]
            h2_t = aux_t[0:32, _C_H2:_C_H2 + 96]
            s8_t = aux_t[0:8, _C_S8:_C_S8 + 11]
            s3_t = aux_t[0:3, _C_S3:_C_S3 + 11]
            sel88_t = aux_t[0:K, _C_SEL88:_C_SEL88 + 88]
            j96_t = aux_t[0:96, _C_J96:_C_J96 + 88]
            oh11_t = aux_t[0:88, _C_OH11:_C_OH11 + 11]
            oh8_t = aux_t[0:88, _C_OH8:_C_OH8 + 8]
            rb88_t = aux_t[0:88, _C_RB88:_C_RB88 + 1]
            km_t = aux_t[0:88, _C_KM:_C_KM + 8]

            # ---- hm loads first: they gate the serial argmax chain ----
            T1, T2 = [], []
            t1_insts = []
            for g in range(G):
                b, s = divmod(g, S)
                t = hmql.tile([128, 1024], f32, name=f"T1_{g}")
                t1_insts.append(nc.sync.dma_start(t[:], t1_src("hm", b, s)))
                T1.append(t)
            _t2q = _os.environ.get("K2T2Q", "sync")
            for g in range(G):
                b, s = divmod(g, S)
                t = hmql.tile([96, 512], f32, name=f"T2_{g}")
                (nc.gpsimd if _t2q == "gp" else nc.sync).dma_start(
                    t[:], t2_src("hm", b, s))
                T2.append(t)

            # ht (SWDGE inline f32->bf16): delayed behind the first T1 loads so
            # the hm stream keeps full HBM bandwidth early
            hT1, hT2 = [], []
            for b in range(BL):
                t1h = sb.tile([128, 1024], bf16, name=f"hT1_{b}")
                ih = nc.gpsimd.dma_start(t1h[:], t1_src("ht", b, None))
                _htd = int(_os.environ.get("K2HTD", "0"))
                if _htd >= 0:
                    tile.add_dep_helper(ih.ins, t1_insts[_htd].ins, sync=False,
                                        reason="delay ht behind early T1 loads")
                hT1.append(t1h)
            for b in range(BL):
                t2h = sb.tile([96, 512], bf16, name=f"hT2_{b}")
                nc.gpsimd.dma_start(t2h[:], t2_src("ht", b, None))
                hT2.append(t2h)

            # ---- small loads ----
            lblr = sb.tile([88, 11], f32, name="lblr")
            for b in range(BL):
                nc.sync.dma_start(
                    lblr[b * S * K:(b + 1) * S * K, :],
                    APc(lbl.tensor, b * K * 11, [[0, S], [11, K], [1, 11]]),
                )
            pred9 = sb.tile([88, 9], f32, name="pred9")
            nc.sync.dma_start(pred9[:], cp[:, :, K:C2, 0, 0:9])

            zrow = sb.tile([1, 512], bf16, name="zrow")
            nc.vector.memset(zrow[:], 0.0)

            # ---- per-group T1: rowmax (DVE) + cast*(-1/2) (ACT) ----
            R1all = sb.tile([128, 64], f32, name="R1all")
            R2all = sb.tile([96, 32], f32, name="R2all")
            C1, C2t = [], []
            do_rmax = "rowmax" not in _skip
            if not do_rmax:
                nc.vector.memset(R1all[:], 0.0)
                nc.vector.memset(R2all[:], 0.0)
            for g in range(G):
                if do_rmax:
                    nc.vector.tensor_reduce(
                        out=R1all[:, 8 * g:8 * (g + 1)],
                        in_=T1[g][:].rearrange("p (j w) -> p j w", j=8),
                        axis=Ax.X, op=Alu.max,
                    )
                c1 = (cst.tile([128, 1024], bf16, name=f"C1_{g}")
                      if _os.environ.get("K2CPOOL", "cst") == "cst"
                      else scr.tile([128, 1024], bf16, name=f"C1_{g}", tag="c1"))
                if "cast" not in _skip:
                    nc.scalar.mul(c1[:], T1[g][:], -0.5)
                C1.append(c1)
            for g in range(G):
                c2 = (cst.tile([96, 512], bf16, name=f"C2_{g}")
                      if _os.environ.get("K2CPOOL", "cst") == "cst"
                      else scr.tile([96, 512], bf16, name=f"C2_{g}", tag="c2"))
                if "cast" not in _skip:
                    nc.scalar.mul(c2[:], T2[g][:], -0.5)
                C2t.append(c2)

            # ---- label-tail prep that only needs lblr/pred9 (cheap, early) ----
            cdiff = sb.tile([88, 7], f32, name="cdiff")
            nc.vector.tensor_tensor(
                out=cdiff[:], in0=pred9[:, 0:7], in1=lblr[:, 0:7], op=Alu.subtract)
            t1 = sb.tile([88, 1], f32, name="t1")
            nc.vector.tensor_tensor(t1[:], lblr[:, 9:10], lblr[:, 7:8], Alu.add)
            t3 = sb.tile([88, 1], f32, name="t3")
            nc.vector.tensor_tensor(t3[:], lblr[:, 10:11], lblr[:, 8:9], Alu.add)
            gmin = sb.tile([88, 1], f32, name="gmin")
            nc.vector.tensor_tensor(gmin[:], lblr[:, 9:10], lblr[:, 10:11], Alu.min)
            gmax = sb.tile([88, 1], f32, name="gmax")
            nc.vector.tensor_tensor(gmax[:], lblr[:, 9:10], lblr[:, 10:11], Alu.max)
            c1v = sb.tile([88, 1], f32, name="c1v")
            nc.vector.tensor_scalar(c1v[:], gmin[:], 0.0, None, Alu.is_gt)
            c2v = sb.tile([88, 1], f32, name="c2v")
            nc.vector.tensor_scalar(c2v[:], gmax[:], float(H), None, Alu.is_lt)
            vv = sb.tile([88, 1], f32, name="vv")
            nc.vector.tensor_tensor(vv[:], c1v[:], c2v[:], Alu.mult)

            # ---- heat-loss matmuls: psall_g += (1/4)hm^2 - (1/2)hm*ht ----
            do_mm = "mm" not in _skip
            psA = ps.tile([128, 512], f32, name="psA", tag="psA")
            psB = ps.tile([128, 512], f32, name="psB", tag="psB")
            nc.tensor.matmul(out=psA[:], lhsT=zrow[:, 0:128], rhs=zrow[:],
                             start=True, stop=False, skip_group_check=True)
            nc.tensor.matmul(out=psB[:], lhsT=zrow[:, 0:128], rhs=zrow[:],
                             start=True, stop=False, skip_group_check=True)
            pht = ps.tile([128, 256], f32, name="pht", tag="pht")
            nc.tensor.matmul(out=pht[:], lhsT=zrow[:, 0:128], rhs=zrow[:, 0:256],
                             start=True, stop=False, skip_group_check=True)

            def ps_g(g):
                t = psA if g < 4 else psB
                return t[:, 128 * (g % 4):128 * (g % 4 + 1)]

            def group_mms(g):
                b = g // S
                first = None
                for c in range(8):
                    sl = slice(128 * c, 128 * (c + 1))
                    i0 = nc.tensor.matmul(out=ps_g(g), lhsT=C1[g][:, sl],
                                          rhs=C1[g][:, sl],
                                          start=False, stop=False,
                                          skip_group_check=True)
                    if first is None:
                        first = i0
                    nc.tensor.matmul(out=ps_g(g), lhsT=hT1[b][:, sl],
                                     rhs=C1[g][:, sl],
                                     start=False, stop=False,
                                     skip_group_check=True)
                for c in range(4):
                    sl = slice(128 * c, 128 * (c + 1))
                    nc.tensor.matmul(out=ps_g(g), lhsT=C2t[g][:, sl],
                                     rhs=C2t[g][:, sl],
                                     start=False, stop=False,
                                     skip_group_check=True)
                    nc.tensor.matmul(out=ps_g(g), lhsT=hT2[b][:, sl],
                                     rhs=C2t[g][:, sl],
                                     start=False, stop=(c == 3),
                                     skip_group_check=True)
                return first

            acc_hm = sb.tile([128, G], f32, name="acc_hm")
            acc_ht = sb.tile([128, BL], f32, name="acc_ht")

            def diag_g(g):
                dsc = scr.tile([128, 128], f32, name=f"dsc{g}", tag="dsc")
                return nc.vector.scalar_tensor_tensor(
                    out=dsc[:], in0=ps_g(g), scalar=1.0, in1=id_t[:],
                    op0=Alu.bypass, op1=Alu.mult, accum_out=acc_hm[:, g:g + 1],
                )

            for g in range(6 if do_mm else 0):
                group_mms(g)

            for b in range(BL if do_mm else 0):
                for c in range(8):
                    nc.tensor.matmul(
                        out=pht[:, 128 * b:128 * (b + 1)],
                        lhsT=hT1[b][:, 128 * c:128 * (c + 1)],
                        rhs=hT1[b][:, 128 * c:128 * (c + 1)],
                        start=False, stop=False, skip_group_check=True,
                    )
                for c in range(4):
                    nc.tensor.matmul(
                        out=pht[:, 128 * b:128 * (b + 1)],
                        lhsT=hT2[b][:, 128 * c:128 * (c + 1)],
                        rhs=hT2[b][:, 128 * c:128 * (c + 1)],
                        start=False, stop=(b == BL - 1 and c == 3),
                        skip_group_check=True,
                    )


            # ---- R2 rowmaxes (all groups, ahead of any head DVE ops) ----
            for g in range(G if do_rmax else 0):
                nc.vector.tensor_reduce(
                    out=R2all[:, 4 * g:4 * (g + 1)],
                    in_=T2[g][:].rearrange("p (j w) -> p j w", j=4),
                    axis=Ax.X, op=Alu.max,
                )

            # ---- T1-part of the argmax head (overlaps late T2 loads) ----
            head1 = ps.tile([128, 512], f32, name="head1", tag="head1")
            head2 = head1
            do_head = "head" not in _skip
            rt1 = head1[0:64, 0:128]
            MX1 = sb.tile([8, 8], f32, name="MX1")    # plane max [k<8, g]
            MX2 = sb.tile([3, 8], f32, name="MX2")    # plane max [k-8, g]
            xp = sb.tile([96, 11], f32, name="xp")    # cols 0:8 T1-k, 8:11 T2-k
            nc.vector.memset(xp[:], 0.0)
            if do_head:
                nc.tensor.transpose(out=rt1, in_=R1all[:], identity=id_t[:])
                rt1s = sb.tile([64, 128], f32, name="rt1s")
                nc.scalar.copy(rt1s[:], rt1)
                V1 = sb.tile([64, 8], f32, name="V1")
                nc.vector.tensor_reduce(
                    out=V1[:], in_=rt1.rearrange("p (k hb) -> p k hb", k=8),
                    axis=Ax.X, op=Alu.max)
                vt1 = head1[0:8, 128:192]
                nc.tensor.transpose(out=vt1, in_=V1[:], identity=id_t[0:64, 0:64])
                nc.vector.tensor_reduce(
                    out=MX1[:], in_=vt1.rearrange("p (g j) -> p g j", g=8),
                    axis=Ax.X, op=Alu.max)
                m1rep = sb.tile([8, 64], f32, name="m1rep")
                nc.vector.tensor_copy(
                    out=m1rep[:].rearrange("k (g j) -> k g j", g=8),
                    in_=MX1[:].rearrange("k (g o) -> k g o", o=1)
                        .to_broadcast([8, 8, 8]),
                )
                mbc1 = head1[0:64, 224:352]
                nc.tensor.matmul(out=mbc1, lhsT=m1rep[:], rhs=ib1_t, start=True,
                                 stop=True)
                mk1 = scr.tile([64, 128], f32, name="mk1", tag="mk")
                nc.vector.tensor_tensor(out=mk1[:], in0=rt1s[:], in1=mbc1,
                                        op=Alu.is_ge)
                mh1 = scr.tile([64, 128], f32, name="mh1", tag="mh")
                nc.vector.tensor_tensor(out=mh1[:], in0=mk1[:], in1=h1_t,
                                        op=Alu.mult)
                nc.vector.tensor_reduce(
                    out=xp[0:64, 0:8],
                    in_=mh1[:].rearrange("p (k hb) -> p k hb", k=8),
                    axis=Ax.X, op=Alu.add)

            # ---- late groups: MMs + R2 + T2-head ----
            late_firsts = []
            _mmd = int(_os.environ.get("K2MMD", "6"))
            for g in range(6, 8):
                if do_mm:
                    fi = group_mms(g)
                    if g >= _mmd:
                        late_firsts.append(fi)

            if do_head:
                rt2 = head2[0:32, 0:96]
                nc.tensor.transpose(out=rt2, in_=R2all[:],
                                    identity=id_t[0:96, 0:96])
                rt2s = sb.tile([32, 96], f32, name="rt2s")
                nc.scalar.copy(rt2s[:], rt2)
                V2 = sb.tile([32, 3], f32, name="V2")
                nc.vector.tensor_reduce(
                    out=V2[:], in_=rt2.rearrange("p (k hb) -> p k hb", k=3),
                    axis=Ax.X, op=Alu.max)
                vt2 = head2[0:3, 192:224]
                nc.tensor.transpose(out=vt2, in_=V2[:], identity=id_t[0:32, 0:32])
                nc.vector.tensor_reduce(
                    out=MX2[:], in_=vt2.rearrange("p (g j) -> p g j", g=8),
                    axis=Ax.X, op=Alu.max)
                m2rep = sb.tile([3, 32], f32, name="m2rep")
                nc.vector.tensor_copy(
                    out=m2rep[:].rearrange("k (g j) -> k g j", g=8),
                    in_=MX2[:].rearrange("k (g o) -> k g o", o=1)
                        .to_broadcast([3, 8, 4]),
                )
                mbc2 = head2[0:32, 352:448]
                nc.tensor.matmul(out=mbc2, lhsT=m2rep[:], rhs=ib2_t, start=True,
                                 stop=True)
                mk2 = scr.tile([32, 96], f32, name="mk2", tag="mk")
                nc.vector.tensor_tensor(out=mk2[:], in0=rt2s[:], in1=mbc2,
                                        op=Alu.is_ge)
                mh2 = scr.tile([32, 96], f32, name="mh2", tag="mh")
                nc.vector.tensor_tensor(out=mh2[:], in0=mk2[:], in1=h2_t,
                                        op=Alu.mult)
                nc.vector.tensor_reduce(
                    out=xp[64:96, 8:11],
                    in_=mh2[:].rearrange("p (k hb) -> p k hb", k=3),
                    axis=Ax.X, op=Alu.add)

            # class loss square (ACT; after the head copies so it doesn't
            # block rt1s/rt2s in the ACT queue)
            csc = sb.tile([88, 7], f32, name="csc")
            cls = sb.tile([88, 1], f32, name="cls")
            nc.scalar.activation(out=csc[:], in_=cdiff[:], func=Act.Square,
                                 accum_out=cls[:])

            # ---- scatter M and x directly onto 88 plane-partitions ----
            if not do_head:
                nc.vector.memset(MX1[:], 1.0)
                nc.vector.memset(MX2[:], 1.0)
            outM = head1[0:88, 448:456]
            nc.tensor.matmul(out=outM, lhsT=aux_t[0:8, _C_SEL88:_C_SEL88 + 88],
                             rhs=MX1[:], start=True, stop=False,
                             skip_group_check=True)
            nc.tensor.matmul(out=outM,
                             lhsT=aux_t[0:3, _C_SEL3B:_C_SEL3B + 88],
                             rhs=MX2[:], start=False, stop=True,
                             skip_group_check=True)
            outX = head1[0:88, 456:467]
            outx_inst = nc.tensor.matmul(out=outX, lhsT=j96_t, rhs=xp[:],
                                         start=True, stop=True,
                                         skip_group_check=True)
            for _fi in late_firsts:
                if _fi is not None and do_head:
                    tile.add_dep_helper(_fi.ins, outx_inst.ins, sync=False,
                                        reason="run argmax-head PE ops first")
            MX88 = sb.tile([88, 2], f32, name="MX88")
            sc88a = scr.tile([88, 8], f32, name="sc88a", tag="sc88")
            nc.vector.scalar_tensor_tensor(
                out=sc88a[:], in0=outM, scalar=1.0, in1=oh8_t,
                op0=Alu.bypass, op1=Alu.mult, accum_out=MX88[:, 0:1],
            )
            sc88b = scr.tile([88, 11], f32, name="sc88b", tag="sc88")
            nc.vector.scalar_tensor_tensor(
                out=sc88b[:], in0=outX, scalar=1.0, in1=oh11_t,
                op0=Alu.bypass, op1=Alu.mult, accum_out=MX88[:, 1:2],
            )
            ridu = sb.tile([88, 1], u32, name="ridu")
            ridu_inst = nc.vector.tensor_tensor(
                out=ridu[:], in0=MX88[:, 1:2], in1=rb88_t, op=Alu.add)
            gath = sb.tile([88, 128], f32, name="gath")
            if "gather" in _skip:
                nc.vector.memset(gath[:], 0.0)
            else:
                nc.gpsimd.indirect_dma_start(
                    out=gath[:],
                    out_offset=None,
                    in_=cp.rearrange("b s c h w -> (b s c h) w"),
                    in_offset=bass.IndirectOffsetOnAxis(ap=ridu[:, 0:1], axis=0),
                )
            conf = sb.tile([88, 1], f32, name="conf")
            nc.scalar.activation(out=conf[:], in_=MX88[:, 0:1], func=Act.Square,
                                 bias=1.0, scale=-1.0)

            # ---- diags: late groups deferred into the gather window; early
            # groups + ht left free for the scheduler to slot into gaps ----
            for g in range(G if do_mm else 0):
                d_inst = diag_g(g)
                if g >= int(_os.environ.get("K2DDEF", "4")):
                    tile.add_dep_helper(
                        d_inst.ins, ridu_inst.ins, sync=False,
                        reason="defer diag past argmax head")
            for b in range(BL if do_mm else 0):
                dht = scr.tile([128, 128], f32, name=f"dht{b}", tag="dsc")
                nc.vector.scalar_tensor_tensor(
                    out=dht[:], in0=pht[:, 128 * b:128 * (b + 1)], scalar=1.0,
                    in1=id_t[:], op0=Alu.bypass, op1=Alu.mult,
                    accum_out=acc_ht[:, b:b + 1],
                )
            if not do_mm:
                nc.vector.memset(acc_hm[:], 0.0)
                nc.vector.memset(acc_ht[:], 0.0)

            # ---- heat loss out ----
            acc_fin = sb.tile([128, G], f32, name="acc_fin")
            for b in range(BL):
                nc.vector.scalar_tensor_tensor(
                    out=acc_fin[:, b * S:(b + 1) * S],
                    in0=acc_hm[:, b * S:(b + 1) * S],
                    scalar=4.0,
                    in1=acc_ht[:, b:b + 1].to_broadcast([128, S]),
                    op0=Alu.mult,
                    op1=Alu.add,
                )
            psum_hs = head1[0:1, 467:475]
            nc.tensor.matmul(out=psum_hs, lhsT=on_t[:], rhs=acc_fin[:],
                             start=True, stop=True)
            heat_row = sb.tile([1, G], f32, name="heat_row")
            nc.vector.tensor_copy(out=heat_row[:], in_=psum_hs)
            nc.scalar.dma_start(out_heat, heat_row[:])

            # ---- y + rest of label tail ----
            yf = sb.tile([88, 1], f32, name="yf")
            ysc = scr.tile([88, 128], f32, name="ysc", tag="ysc")
            nc.vector.scalar_tensor_tensor(
                out=ysc[:], in0=gath[:], scalar=MX88[:, 0:1], in1=io_t[0:88, :],
                op0=Alu.is_equal, op1=Alu.mult, accum_out=yf[:],
            )
            t2 = sb.tile([88, 1], f32, name="t2")
            nc.vector.tensor_tensor(t2[:], MX88[:, 1:2], pred9[:, 7:8], Alu.add)
            tx = sb.tile([88, 1], f32, name="tx")
            nc.vector.tensor_tensor(tx[:], t1[:], t2[:], Alu.subtract)
            ccv = sb.tile([88, 1], f32, name="ccv")
            nc.vector.tensor_tensor(ccv[:], cls[:], conf[:], Alu.add)
            base = sb.tile([88, 1], f32, name="base")
            nc.vector.scalar_tensor_tensor(
                out=base[:], in0=tx[:], scalar=tx[:, 0:1], in1=ccv[:],
                op0=Alu.mult, op1=Alu.add,
            )
            t4 = sb.tile([88, 1], f32, name="t4")
            nc.vector.tensor_tensor(t4[:], yf[:], pred9[:, 8:9], Alu.add)
            ty = sb.tile([88, 1], f32, name="ty")
            nc.vector.tensor_tensor(ty[:], t3[:], t4[:], Alu.subtract)
            tot3 = sb.tile([88, 1], f32, name="tot3")
            nc.vector.scalar_tensor_tensor(
                out=tot3[:], in0=ty[:], scalar=ty[:, 0:1], in1=base[:],
                op0=Alu.mult, op1=Alu.add,
            )
            perkp = sb.tile([88, 1], f32, name="perkp")
            nc.vector.tensor_tensor(perkp[:], tot3[:], vv[:], Alu.mult)
            if _dbg:
                nc.sync.dma_start(out_r1, R1all[:])
            psum_lk = head2[0:1, 475:483]
            nc.tensor.matmul(out=psum_lk, lhsT=perkp[:], rhs=km_t, start=True,
                             stop=True)
            lab_row = sb.tile([1, G], f32, name="lab_row")
            nc.vector.tensor_copy(out=lab_row[:], in_=psum_lk)
            nc.sync.dma_start(out_label, lab_row[:])

        unroll = int(__import__("os").environ.get("K2UNROLL", "2"))
        if loop_n > 1 and loop_n >= 2 * unroll and unroll > 1:
            n2 = loop_n // unroll
            rem = loop_n - n2 * unroll
            with tc.For_i(0, n2, 1):
                for _u in range(unroll):
                    emit()
            for _u in range(rem):
                emit()
        elif loop_n > 1:
            with tc.For_i(0, loop_n, 1):
                emit()
        else:
            emit()

    nc.compile()
    return nc


def _get_nc(reps=1, loop_n=1):
    key = f"nc{reps}_{loop_n}"
    if key not in _CACHE:
        _CACHE[key] = _build_module(loop_n)
    return _CACHE[key]


def _in_maps(combined_preds, heatmaps, labels):
    cpv = np.ascontiguousarray(combined_preds, dtype=np.float32)
    hmv = np.ascontiguousarray(heatmaps, dtype=np.float32)
    lbv = np.ascontiguousarray(labels, dtype=np.float32)
    maps = []
    for i in range(NCORES):
        b0 = BL * i
        maps.append(
            {
                "cp": np.ascontiguousarray(cpv[b0:b0 + BL]),
                "hmr": np.ascontiguousarray(hmv[b0:b0 + BL]),
                "lbl": np.ascontiguousarray(lbv[b0:b0 + BL]),
            }
        )
    return maps


def run(combined_preds, heatmaps, labels, trace=False):
    from concourse import bass_utils

    nc = _get_nc()
    res = bass_utils.run_bass_kernel_spmd(
        nc,
        _in_maps(combined_preds, heatmaps, labels),
        core_ids=list(range(NCORES)),
        trace=trace,
    )
    # out_heat/out_label are [1, G] rows with G = BL*S in (b, s) order
    heat = np.concatenate(
        [res.results[i]["out_heat"].reshape(BL, S) for i in range(NCORES)], axis=0
    )
    lab = np.concatenate(
        [res.results[i]["out_label"].reshape(BL, S) for i in range(NCORES)], axis=0
    )
    return (heat, lab), res


def kernel(combined_preds, heatmaps, labels):
    (heat, lab), _ = run(combined_preds, heatmaps, labels)
    return heat, lab
